# revision 21
# baseline (speedup 1.0000x reference)
"""Trainium2 Bass kernel for BERTIdealEmissionRateCompressionModule.

reference math (teacher path):
    head_mean = attentions.mean(axis=2)          # [L, B, S, S]
    prod      = prod_L head_mean                 # [B, S, S]
    y_soft    = -prod[:, 0, :]                   # [B, S]   <- only CLS row used!
    y_hard    = rank(y_soft with y[0]=min-1) < k # [B, S] bool, stable ranking

Only attentions[:, :, :, 0, :] (L*H*S floats per batch row) is live data.
Sharding: pure data parallel over batch B=8 -> one batch row per NeuronCore.

Per-core device pipeline (att [144, 512] f32, 144 = L*H):
  1. DMA att -> SBUF (a1 [128,512], a2 [16,512])
  2. per 128-token segment t: PE-transpose -> attT [128(tok), 144(l,h)]
     reduce_sum groups of 12 -> *1/12 -> head means hmT [128, 12]
     pairwise free-dim mults -> prod_t [128, 1]
  3. PE-transpose prod cols -> prow [1, 512]; y_soft = -prow (DMA out)
  4. psel = prow with psel[0] = max+1 (CLS always kept; == -(min(y)-1))
  5. broadcast psel over partitions via K=1 matmul -> bc [128, 512]
  6. rank[j] = sum_i [psel[i] > psel[j]] + sum_{i<j} [psel[i]==psel[j]]
     via tensor_scalar is_lt/is_equal + affine_select triangle mask,
     summed over i with ones-vector matmuls into PSUM [1, 512]
  7. y_hard = rank < k (DMA out as f32 0/1, cast to bool on host)
"""

import functools

import numpy as np

L, B, H, S = 12, 8, 12, 512
LH = L * H  # 144
N_CORES = 8
SEG = 128
N_SEG = S // SEG  # 4


@functools.lru_cache(maxsize=4)
def _build(k: int):
    import concourse.bass as bass
    import concourse.mybir as mybir
    from concourse.tile import TileContext
    from concourse.vector_clock import ScopedClock

    class TileContextSplitDrain(TileContext):
        """This walrus codegen fits a single embedded sync wait per
        instruction; Tile's kernel-tail drain aggregates one wait per live
        semaphore onto one Drain. Split it into a chain of single-wait
        drains on the sync queue (same semantics: all waits complete
        before the end-of-kernel barrier)."""

        def _drain_and_barrier(self, tick_clock, wait_clock):
            nc = self.nc
            drain_inst = nc.sync.drain()
            wait_clock.add_sem_waits(
                drain_inst.ins, ScopedClock({None: tick_clock.global_clock})
            )
            si = drain_inst.ins.sync_info
            if si is not None and len(si.on_wait) > 1:
                waits = list(si.on_wait)
                ups = list(si.on_update)
                drain_inst.ins.sync_info = mybir.SyncInfo(
                    on_wait=[waits[0]], on_update=[])
                for w in waits[1:-1]:
                    d = nc.sync.drain()
                    d.ins.sync_info = mybir.SyncInfo(on_wait=[w], on_update=[])
                d = nc.sync.drain()
                d.ins.sync_info = mybir.SyncInfo(
                    on_wait=[waits[-1]], on_update=ups)
            nc.all_engine_barrier()
            assert self.sems is not None
            popped = nc._tile_sem_poison_stack.pop()
            assert popped is self._sem_poison
            nc.clear_and_free_semaphores(list(self.sems.allocated().values()))
            nc.all_engine_barrier()

    f32 = mybir.dt.float32
    Alu = mybir.AluOpType
    X = mybir.AxisListType.X

    nc = bass.Bass()
    att = nc.declare_dram_parameter("att", [LH, S], f32, isOutput=False)
    y_soft = nc.dram_tensor("y_soft", [1, S], f32, kind="ExternalOutput")
    # y_hard in column form: y_hard_col[p, t] = mask[128*t + p]
    y_hard = nc.dram_tensor("y_hard", [SEG, N_SEG], f32, kind="ExternalOutput")

    # one packed const -> one DMA -> one wait semaphore for the PE
    # [:, 0:128] = id128 (id16 is its top-left slice); [0, 128:256] = ones row
    pack = np.zeros((128, 256), dtype=np.float32)
    pack[:, 0:128] = np.eye(128, dtype=np.float32)
    pack[0, 128:256] = 1.0
    pack_d = nc.inline_tensor(pack, "cpack")

    # triangle masks, tri[p, t*S + f] = 1.0 iff f < 128*t + p  (i.e. i < j)
    i_idx = np.arange(S)[None, :]
    tri = np.concatenate(
        [(i_idx < (t * SEG + np.arange(SEG)[:, None])) for t in range(N_SEG)],
        axis=1,
    ).astype(np.float32)
    tri_d = nc.inline_tensor(tri, "ctri")

    with TileContextSplitDrain(nc) as tc:
        with (
            tc.tile_pool(name="const", bufs=1) as cpool,
            tc.tile_pool(name="inp", bufs=1) as ipool,
            tc.tile_pool(name="work", bufs=2) as wpool,
            tc.tile_pool(name="prodc", bufs=N_SEG) as ppool,
            tc.tile_pool(name="rowbuf", bufs=1) as rpool,
            tc.tile_pool(name="cmp", bufs=3) as mpool,
            tc.tile_pool(name="pT", bufs=N_SEG, space="PSUM") as pT_pool,
            tc.tile_pool(name="prow", bufs=1, space="PSUM") as prow_pool,
            tc.tile_pool(name="pbc", bufs=1, space="PSUM") as pbc_pool,
            tc.tile_pool(name="pdum", bufs=1, space="PSUM") as pdum_pool,
        ):
            cpack = cpool.tile([128, 256], f32, tag="cpack")
            nc.sync.dma_start(cpack[:], pack_d[:])
            id128 = cpack[:, 0:128]
            id16 = cpack[0:16, 0:16]
            ones_r = cpack[0:1, 128:256]
            ctri = cpool.tile([128, N_SEG * S], f32, tag="ctri")
            nc.sync.dma_start(ctri[:], tri_d[:])

            a1 = ipool.tile([128, S], f32, tag="a1")
            nc.sync.dma_start(a1[:], att[0:128, :])
            a2 = ipool.tile([16, S], f32, tag="a2")
            nc.sync.dma_start(a2[:], att[128:144, :])

            # dummy 1x1 ops: make PE / DVE take their const-DMA waits alone,
            # so every later instruction carries at most one sync wait
            # (this walrus codegen fits a single wait per compute inst).
            pdum = pdum_pool.tile([1, 1], f32, tag="pdum")
            nc.tensor.transpose(pdum[:], cpack[0:1, 0:1], cpack[0:1, 0:1])
            vdum = rpool.tile([1, 1], f32, tag="vdum")
            nc.vector.tensor_copy(vdum[:], ctri[0:1, 0:1])

            psum_row = prow_pool.tile([1, S], f32, tag="prow")
            prods = []
            for t in range(N_SEG):
                seg = slice(t * SEG, (t + 1) * SEG)
                pT = pT_pool.tile([128, LH], f32, tag="pT")
                nc.tensor.transpose(pT[:, 0:128], a1[:, seg], id128)
                nc.tensor.transpose(pT[:, 128:144], a2[:, seg], id16)

                sums = wpool.tile([128, L], f32, tag="sums")
                nc.vector.tensor_reduce(
                    sums[:], pT[:].rearrange("p (l h) -> p l h", h=H), axis=X,
                    op=Alu.add,
                )
                hmT = wpool.tile([128, L], f32, tag="hmT")
                nc.vector.tensor_scalar_mul(hmT[:], sums[:], float(np.float32(1.0 / 12.0)))

                p6 = wpool.tile([128, 6], f32, tag="p6")
                nc.vector.tensor_tensor(p6[:], hmT[:, 0:6], hmT[:, 6:12], op=Alu.mult)
                p3 = wpool.tile([128, 3], f32, tag="p3")
                nc.vector.tensor_tensor(p3[:], p6[:, 0:3], p6[:, 3:6], op=Alu.mult)
                p1 = wpool.tile([128, 1], f32, tag="p1")
                nc.vector.tensor_tensor(p1[:], p3[:, 0:1], p3[:, 1:2], op=Alu.mult)
                prod_t = ppool.tile([128, 1], f32, tag="prod")
                nc.vector.tensor_tensor(prod_t[:], p1[:], p3[:, 2:3], op=Alu.mult)
                prods.append(prod_t)

                nc.tensor.transpose(psum_row[0:1, seg], prod_t[:], id128)

            # y_soft = -prod (original, no CLS replacement)
            ysoft_s = rpool.tile([1, S], f32, tag="ysoft")
            nc.vector.tensor_scalar_mul(ysoft_s[:], psum_row[:], -1.0)
            nc.sync.dma_start(y_soft[:], ysoft_s[:])

            # psel: prod row with psel[0] = max(prod) + 1
            psel = rpool.tile([1, S], f32, tag="psel")
            nc.vector.tensor_copy(psel[:], psum_row[:])
            m = rpool.tile([1, 1], f32, tag="m")
            nc.vector.tensor_reduce(m[:], psel[:], axis=X, op=Alu.max)
            m1 = rpool.tile([1, 1], f32, tag="m1")
            nc.vector.tensor_scalar_add(m1[:], m[:], 1.0)
            nc.vector.tensor_copy(psel[0:1, 0:1], m1[:])

            # column form of psel: only segment 0 differs from prod_t tiles
            pcol0 = rpool.tile([128, 1], f32, tag="pcol0")
            nc.vector.tensor_copy(pcol0[:], prods[0][:])
            nc.vector.tensor_copy(pcol0[0:1, 0:1], m1[:])
            cols = [pcol0] + prods[1:]

            # broadcast psel across partitions: bc[p, i] = psel[i]
            psum_bc = pbc_pool.tile([128, S], f32, tag="bc")
            nc.tensor.matmul(psum_bc[:], ones_r, psel[:])
            bc = rpool.tile([128, S], f32, tag="bcs")
            nc.vector.tensor_copy(bc[:], psum_bc[:])

            # rank[j] = #{i: psel[i] > psel[j]} + #{i<j: psel[i]==psel[j]}
            # partition p = j-within-segment (j = 128t+p), free f = i
            hardcol = rpool.tile([128, N_SEG], f32, tag="hardcol")
            for t in range(N_SEG):
                col = cols[t]
                gt = mpool.tile([128, S], f32, tag="gt")
                nc.vector.tensor_scalar(
                    gt[:], bc[:], col[:], None, op0=Alu.is_gt)
                eq = mpool.tile([128, S], f32, tag="eq")
                nc.vector.tensor_scalar(
                    eq[:], bc[:], col[:], None, op0=Alu.is_equal)
                # keep eq only where i (=f) < j (=128t+p)
                eqm = mpool.tile([128, S], f32, tag="eqm")
                nc.vector.tensor_tensor(
                    eqm[:], eq[:], ctri[:, t * S:(t + 1) * S], op=Alu.mult)
                s_t = mpool.tile([128, S], f32, tag="s")
                nc.vector.tensor_tensor(s_t[:], gt[:], eqm[:], op=Alu.add)
                rank_t = wpool.tile([128, 1], f32, tag="rank")
                nc.vector.tensor_reduce(rank_t[:], s_t[:], axis=X, op=Alu.add)
                nc.vector.tensor_scalar(
                    hardcol[:, t:t + 1], rank_t[:], float(k), None,
                    op0=Alu.is_lt)

            nc.sync.dma_start(y_hard[:], hardcol[:])

    return nc


LAST_RESULT = None  # BassKernelResults of the most recent run (for profiling)


def _ensure_ntff_hook():
    """bass_utils hard-imports antenv.axon_hooks when tracing is requested;
    this container's antenv lacks it. Provide it (with a working hook when
    the axon .so supports NRT profiling)."""
    import sys
    import types

    try:
        import antenv.axon_hooks  # noqa: F401

        return
    except ImportError:
        pass
    mod = types.ModuleType("antenv.axon_hooks")
    state = [None]
    mod.set_axon_ntff_profile_hook = lambda h: state.__setitem__(0, h)
    mod.get_axon_ntff_profile_hook = lambda: state[0]
    try:
        from trn_agent_boot.trn_boot import _ntff_profile_via_ctypes

        state[0] = _ntff_profile_via_ctypes("/opt/axon/libaxon_pjrt.so")
    except Exception:
        pass
    try:
        import antenv

        antenv.axon_hooks = mod
    except ImportError:
        pass
    sys.modules["antenv.axon_hooks"] = mod


def _run(att_cls: np.ndarray, k: int):
    global LAST_RESULT
    _ensure_ntff_hook()
    from concourse.bass_utils import run_bass_kernel_spmd

    nc = _build(k)
    in_maps = [
        {"att": np.ascontiguousarray(att_cls[:, b].reshape(LH, S))}
        for b in range(B)
    ]
    LAST_RESULT = run_bass_kernel_spmd(nc, in_maps, list(range(N_CORES)))
    res = LAST_RESULT.results
    y_soft = np.stack([res[b]["y_soft"][0] for b in range(B)])
    y_hard = np.stack([res[b]["y_hard"].T.reshape(S) for b in range(B)]) > 0.5
    return y_hard, y_soft


def kernel(attentions, embedding_sequence, compression_rate):
    att = np.asarray(attentions)
    seq_len = int(np.asarray(embedding_sequence).shape[1])
    k = max(int(seq_len * (1.0 - float(np.asarray(compression_rate)))), 1)
    att_cls = np.ascontiguousarray(att[:, :, :, 0, :], dtype=np.float32)
    y_hard, y_soft = _run(att_cls, k)
    return y_hard, y_soft


# revision 26
# speedup vs baseline: 1.3397x; 1.3397x over previous
"""Trainium2 Bass kernel for BERTIdealEmissionRateCompressionModule.

reference math (teacher path):
    head_mean = attentions.mean(axis=2)          # [L, B, S, S]
    prod      = prod_L head_mean                 # [B, S, S]
    y_soft    = -prod[:, 0, :]                   # [B, S]   <- only CLS row used!
    y_hard    = rank(y_soft with y[0]=min-1) < k # [B, S] bool, stable ranking

Only attentions[:, :, :, 0, :] (L*H*S floats per batch row) is live data.
Sharding: pure data parallel over batch B=8 -> one batch row per NeuronCore.

Per-core device pipeline (att [144, 512] f32, 144 = L*H):
  1. DMA att -> SBUF (a1 [128,512], a2 [16,512])
  2. per 128-token segment t: PE-transpose -> attT [128(tok), 144(l,h)]
     reduce_sum groups of 12 -> *1/12 -> head means hmT [128, 12]
     pairwise free-dim mults -> prod_t [128, 1]
  3. PE-transpose prod cols -> prow [1, 512]; y_soft = -prow (DMA out)
  4. psel = prow with psel[0] = max+1 (CLS always kept; == -(min(y)-1))
  5. broadcast psel over partitions via K=1 matmul -> bc [128, 512]
  6. rank[j] = sum_i [psel[i] > psel[j]] + sum_{i<j} [psel[i]==psel[j]]
     via tensor_scalar is_lt/is_equal + affine_select triangle mask,
     summed over i with ones-vector matmuls into PSUM [1, 512]
  7. y_hard = rank < k (DMA out as f32 0/1, cast to bool on host)
"""

import functools

import numpy as np

L, B, H, S = 12, 8, 12, 512
LH = L * H  # 144
N_CORES = 8
SEG = 128
N_SEG = S // SEG  # 4


@functools.lru_cache(maxsize=4)
def _build(k: int):
    import concourse.bass as bass
    import concourse.mybir as mybir
    from concourse.tile import TileContext
    from concourse.vector_clock import ScopedClock

    class TileContextSplitDrain(TileContext):
        """This walrus codegen fits a single embedded sync wait per
        instruction; Tile's kernel-tail drain aggregates one wait per live
        semaphore onto one Drain. Split it into a chain of single-wait
        drains on the sync queue (same semantics: all waits complete
        before the end-of-kernel barrier)."""

        def _drain_and_barrier(self, tick_clock, wait_clock):
            nc = self.nc
            drain_inst = nc.sync.drain()
            wait_clock.add_sem_waits(
                drain_inst.ins, ScopedClock({None: tick_clock.global_clock})
            )
            si = drain_inst.ins.sync_info
            if si is not None and len(si.on_wait) > 1:
                waits = list(si.on_wait)
                ups = list(si.on_update)
                drain_inst.ins.sync_info = mybir.SyncInfo(
                    on_wait=[waits[0]], on_update=[])
                for w in waits[1:-1]:
                    d = nc.sync.drain()
                    d.ins.sync_info = mybir.SyncInfo(on_wait=[w], on_update=[])
                d = nc.sync.drain()
                d.ins.sync_info = mybir.SyncInfo(
                    on_wait=[waits[-1]], on_update=ups)
            nc.all_engine_barrier()
            assert self.sems is not None
            popped = nc._tile_sem_poison_stack.pop()
            assert popped is self._sem_poison
            nc.clear_and_free_semaphores(list(self.sems.allocated().values()))
            nc.all_engine_barrier()

    f32 = mybir.dt.float32
    Alu = mybir.AluOpType
    X = mybir.AxisListType.X

    nc = bass.Bass()
    att = nc.declare_dram_parameter("att", [LH, S], f32, isOutput=False)
    y_soft = nc.dram_tensor("y_soft", [1, S], f32, kind="ExternalOutput")
    # y_hard in column form: y_hard_col[p, t] = mask[128*t + p]
    y_hard = nc.dram_tensor("y_hard", [SEG, N_SEG], f32, kind="ExternalOutput")

    # one packed const -> one DMA -> one wait semaphore for the PE
    # [:, 0:128] = id128 (id16 is its top-left slice); [0, 128:256] = ones row
    pack = np.zeros((128, 256), dtype=np.float32)
    pack[:, 0:128] = np.eye(128, dtype=np.float32)
    pack[0, 128:256] = 1.0
    pack_d = nc.inline_tensor(pack, "cpack")



    with TileContextSplitDrain(nc) as tc:
        with (
            tc.tile_pool(name="const", bufs=1) as cpool,
            tc.tile_pool(name="inp", bufs=1) as ipool,
            tc.tile_pool(name="work", bufs=2) as wpool,
            tc.tile_pool(name="prodc", bufs=N_SEG) as ppool,
            tc.tile_pool(name="rowbuf", bufs=1) as rpool,
            tc.tile_pool(name="cmp", bufs=3) as mpool,
            tc.tile_pool(name="pT", bufs=N_SEG, space="PSUM") as pT_pool,
            tc.tile_pool(name="prow", bufs=1, space="PSUM") as prow_pool,
            tc.tile_pool(name="pbc", bufs=1, space="PSUM") as pbc_pool,
            tc.tile_pool(name="pdum", bufs=1, space="PSUM") as pdum_pool,
        ):
            cpack = cpool.tile([128, 256], f32, tag="cpack")
            nc.sync.dma_start(cpack[:], pack_d[:])
            id128 = cpack[:, 0:128]
            id16 = cpack[0:16, 0:16]
            ones_r = cpack[0:1, 128:256]

            a1 = ipool.tile([128, S], f32, tag="a1")
            nc.sync.dma_start(a1[:], att[0:128, :])
            a2 = ipool.tile([16, S], f32, tag="a2")
            nc.sync.dma_start(a2[:], att[128:144, :])

            # dummy 1x1 ops: make PE / DVE take their const-DMA waits alone,
            # so every later instruction carries at most one sync wait
            # (this walrus codegen fits a single wait per compute inst).
            pdum = pdum_pool.tile([1, 1], f32, tag="pdum")
            nc.tensor.transpose(pdum[:], cpack[0:1, 0:1], cpack[0:1, 0:1])

            psum_row = prow_pool.tile([1, S], f32, tag="prow")
            prods = []
            for t in range(N_SEG):
                seg = slice(t * SEG, (t + 1) * SEG)
                pT = pT_pool.tile([128, LH], f32, tag="pT")
                nc.tensor.transpose(pT[:, 0:128], a1[:, seg], id128)
                nc.tensor.transpose(pT[:, 128:144], a2[:, seg], id16)

                sums = wpool.tile([128, L], f32, tag="sums")
                nc.vector.tensor_reduce(
                    sums[:], pT[:].rearrange("p (l h) -> p l h", h=H), axis=X,
                    op=Alu.add,
                )
                hmT = wpool.tile([128, L], f32, tag="hmT")
                nc.vector.tensor_scalar_mul(hmT[:], sums[:], float(np.float32(1.0 / 12.0)))

                p6 = wpool.tile([128, 6], f32, tag="p6")
                nc.vector.tensor_tensor(p6[:], hmT[:, 0:6], hmT[:, 6:12], op=Alu.mult)
                p3 = wpool.tile([128, 3], f32, tag="p3")
                nc.vector.tensor_tensor(p3[:], p6[:, 0:3], p6[:, 3:6], op=Alu.mult)
                p1 = wpool.tile([128, 1], f32, tag="p1")
                nc.vector.tensor_tensor(p1[:], p3[:, 0:1], p3[:, 1:2], op=Alu.mult)
                prod_t = ppool.tile([128, 1], f32, tag="prod")
                nc.vector.tensor_tensor(prod_t[:], p1[:], p3[:, 2:3], op=Alu.mult)
                prods.append(prod_t)

                nc.tensor.transpose(psum_row[0:1, seg], prod_t[:], id128)

            # y_soft = -prod (original, no CLS replacement)
            ysoft_s = rpool.tile([1, S], f32, tag="ysoft")
            nc.vector.tensor_scalar_mul(ysoft_s[:], psum_row[:], -1.0)
            nc.sync.dma_start(y_soft[:], ysoft_s[:])

            # psel: prod row with psel[0] = max(prod) + 1
            psel = rpool.tile([1, S], f32, tag="psel")
            nc.vector.tensor_copy(psel[:], psum_row[:])
            m = rpool.tile([1, 1], f32, tag="m")
            nc.vector.tensor_reduce(m[:], psel[:], axis=X, op=Alu.max)
            m1 = rpool.tile([1, 1], f32, tag="m1")
            nc.vector.tensor_scalar_add(m1[:], m[:], 1.0)
            nc.vector.tensor_copy(psel[0:1, 0:1], m1[:])

            # column form of psel: only segment 0 differs from prod_t tiles
            pcol0 = rpool.tile([128, 1], f32, tag="pcol0")
            nc.vector.tensor_copy(pcol0[:], prods[0][:])
            nc.vector.tensor_copy(pcol0[0:1, 0:1], m1[:])
            cols = [pcol0] + prods[1:]

            # broadcast psel across partitions: bc[p, i] = psel[i]
            psum_bc = pbc_pool.tile([128, S], f32, tag="bc")
            nc.tensor.matmul(psum_bc[:], ones_r, psel[:])
            bc = rpool.tile([128, S], f32, tag="bcs")
            nc.vector.tensor_copy(bc[:], psum_bc[:])

            # strict rank[j] = #{i: psel[i] > psel[j]}; partition p = j-within-
            # segment (j = 128t+p), free f = i.  Exact ties (impossible for
            # real attention products) fall back to a host recompute, keyed
            # off the y_soft output.
            hardcol = rpool.tile([128, N_SEG], f32, tag="hardcol")
            for t in range(N_SEG):
                col = cols[t]
                gt = mpool.tile([128, S], f32, tag="gt")
                rank_t = wpool.tile([128, 1], f32, tag="rank")
                nc.vector.tensor_scalar(
                    gt[:], bc[:], col[:], None, op0=Alu.is_gt, op1=Alu.add,
                    accum_out=rank_t[:])
                nc.vector.tensor_scalar(
                    hardcol[:, t:t + 1], rank_t[:], float(k), None,
                    op0=Alu.is_lt)

            nc.sync.dma_start(y_hard[:], hardcol[:])

    return nc


LAST_RESULT = None  # BassKernelResults of the most recent run (for profiling)


def _ensure_ntff_hook():
    """bass_utils hard-imports antenv.axon_hooks when tracing is requested;
    this container's antenv lacks it. Provide it (with a working hook when
    the axon .so supports NRT profiling)."""
    import sys
    import types

    try:
        import antenv.axon_hooks  # noqa: F401

        return
    except ImportError:
        pass
    mod = types.ModuleType("antenv.axon_hooks")
    state = [None]
    mod.set_axon_ntff_profile_hook = lambda h: state.__setitem__(0, h)
    mod.get_axon_ntff_profile_hook = lambda: state[0]
    try:
        from trn_agent_boot.trn_boot import _ntff_profile_via_ctypes

        state[0] = _ntff_profile_via_ctypes("/opt/axon/libaxon_pjrt.so")
    except Exception:
        pass
    try:
        import antenv

        antenv.axon_hooks = mod
    except ImportError:
        pass
    sys.modules["antenv.axon_hooks"] = mod


def _run(att_cls: np.ndarray, k: int):
    global LAST_RESULT
    _ensure_ntff_hook()
    from concourse.bass_utils import run_bass_kernel_spmd

    nc = _build(k)
    in_maps = [
        {"att": np.ascontiguousarray(att_cls[:, b].reshape(LH, S))}
        for b in range(B)
    ]
    LAST_RESULT = run_bass_kernel_spmd(nc, in_maps, list(range(N_CORES)))
    res = LAST_RESULT.results
    y_soft = np.stack([res[b]["y_soft"][0] for b in range(B)])
    y_hard = np.stack([res[b]["y_hard"].T.reshape(S) for b in range(B)]) > 0.5
    if any(np.unique(y_soft[b]).size != S for b in range(B)):
        # exact duplicate values: strict rank != stable rank; replicate the
        # reference's stable double-argsort on host (f32, global min)
        y = y_soft.copy()
        y[:, 0] = np.float32(y_soft.min() - np.float32(1.0))
        order = np.argsort(y, axis=-1, kind="stable")
        rank = np.argsort(order, axis=-1, kind="stable")
        y_hard = rank < k
    return y_hard, y_soft


def kernel(attentions, embedding_sequence, compression_rate):
    att = np.asarray(attentions)
    seq_len = int(np.asarray(embedding_sequence).shape[1])
    k = max(int(seq_len * (1.0 - float(np.asarray(compression_rate)))), 1)
    att_cls = np.ascontiguousarray(att[:, :, :, 0, :], dtype=np.float32)
    y_hard, y_soft = _run(att_cls, k)
    return y_hard, y_soft


# revision 32
# speedup vs baseline: 1.7063x; 1.2736x over previous
"""Trainium2 Bass kernel for BERTIdealEmissionRateCompressionModule.

reference math (teacher path):
    head_mean = attentions.mean(axis=2)          # [L, B, S, S]
    prod      = prod_L head_mean                 # [B, S, S]
    y_soft    = -prod[:, 0, :]                   # [B, S]   <- only CLS row used!
    y_hard    = rank(y_soft with y[0]=min-1) < k # [B, S] bool, stable ranking

Only attentions[:, :, :, 0, :] (L*H*S floats per batch row) is live data.
Sharding: pure data parallel over batch B=8 -> one batch row per NeuronCore.

Per-core device pipeline (att [144, 512] f32, 144 = L*H):
  1. DMA att -> SBUF (a1 [128,512], a2 [16,512])
  2. per 128-token segment t: PE-transpose -> attT [128(tok), 144(l,h)]
     reduce_sum groups of 12 -> *1/12 -> head means hmT [128, 12]
     pairwise free-dim mults -> prod_t [128, 1]
  3. PE-transpose prod cols -> prow [1, 512]; y_soft = -prow (DMA out)
  4. psel = prow with psel[0] = max+1 (CLS always kept; == -(min(y)-1))
  5. broadcast psel over partitions via K=1 matmul -> bc [128, 512]
  6. rank[j] = sum_i [psel[i] > psel[j]] + sum_{i<j} [psel[i]==psel[j]]
     via tensor_scalar is_lt/is_equal + affine_select triangle mask,
     summed over i with ones-vector matmuls into PSUM [1, 512]
  7. y_hard = rank < k (DMA out as f32 0/1, cast to bool on host)
"""

import functools

import numpy as np

L, B, H, S = 12, 8, 12, 512
LH = L * H  # 144
N_CORES = 8
SEG = 128
N_SEG = S // SEG  # 4


@functools.lru_cache(maxsize=4)
def _build(k: int):
    import concourse.bass as bass
    import concourse.mybir as mybir
    from concourse.tile import TileContext
    from concourse.vector_clock import ScopedClock

    class TileContextSplitDrain(TileContext):
        """This walrus codegen fits a single embedded sync wait per
        instruction; Tile's kernel-tail drain aggregates one wait per live
        semaphore onto one Drain. Split it into a chain of single-wait
        drains on the sync queue (same semantics: all waits complete
        before the end-of-kernel barrier)."""

        def _drain_and_barrier(self, tick_clock, wait_clock):
            nc = self.nc
            drain_inst = nc.sync.drain()
            wait_clock.add_sem_waits(
                drain_inst.ins, ScopedClock({None: tick_clock.global_clock})
            )
            si = drain_inst.ins.sync_info
            if si is not None and len(si.on_wait) > 1:
                waits = list(si.on_wait)
                ups = list(si.on_update)
                drain_inst.ins.sync_info = mybir.SyncInfo(
                    on_wait=[waits[0]], on_update=[])
                for w in waits[1:-1]:
                    d = nc.sync.drain()
                    d.ins.sync_info = mybir.SyncInfo(on_wait=[w], on_update=[])
                d = nc.sync.drain()
                d.ins.sync_info = mybir.SyncInfo(
                    on_wait=[waits[-1]], on_update=ups)
            nc.all_engine_barrier()
            assert self.sems is not None
            popped = nc._tile_sem_poison_stack.pop()
            assert popped is self._sem_poison
            nc.clear_and_free_semaphores(list(self.sems.allocated().values()))
            nc.all_engine_barrier()

    f32 = mybir.dt.float32
    Alu = mybir.AluOpType
    X = mybir.AxisListType.X

    nc = bass.Bass()
    att = nc.declare_dram_parameter("att", [LH, S], f32, isOutput=False)
    # packed column-form output: cols 0:4 = y_soft, cols 4:8 = y_hard mask,
    # out[p, c] for token j = 128*(c%4) + p
    y_out = nc.dram_tensor("y_out", [SEG, 2 * N_SEG], f32,
                           kind="ExternalOutput")

    # one packed const -> one DMA -> one wait semaphore for the PE
    # [:, 0:128] = id128 (id16 is its top-left slice)
    pack = np.eye(128, dtype=np.float32)
    pack_d = nc.inline_tensor(pack, "cpack")



    with TileContextSplitDrain(nc) as tc:
        with (
            tc.tile_pool(name="const", bufs=1) as cpool,
            tc.tile_pool(name="inp", bufs=1) as ipool,
            tc.tile_pool(name="work", bufs=2) as wpool,
            tc.tile_pool(name="rowbuf", bufs=1) as rpool,
            tc.tile_pool(name="cmp", bufs=N_SEG) as mpool,
            tc.tile_pool(name="pT", bufs=N_SEG, space="PSUM") as pT_pool,
            tc.tile_pool(name="prow", bufs=1, space="PSUM") as prow_pool,
            tc.tile_pool(name="pdum", bufs=1, space="PSUM") as pdum_pool,
        ):
            # a1 first on the SP queue so its transfer overlaps the preamble;
            # consts + a2 issue in parallel from the ACT HWDGE queue
            a1 = ipool.tile([128, S], f32, tag="a1")
            nc.sync.dma_start(a1[:], att[0:128, :])
            cpack = cpool.tile([128, 128], f32, tag="cpack")
            nc.scalar.dma_start(cpack[:], pack_d[:])
            id128 = cpack[:, 0:128]
            id16 = cpack[0:16, 0:16]
            a2 = ipool.tile([16, S], f32, tag="a2")
            nc.scalar.dma_start(a2[:], att[128:144, :])

            # dummy 1x1 transpose: makes the PE take the const-DMA wait
            # alone, so every later matmul carries at most one sync wait
            # (this walrus codegen fits a single wait per compute inst).
            pdum = pdum_pool.tile([1, 1], f32, tag="pdum")
            nc.tensor.transpose(pdum[:], cpack[0:1, 0:1], cpack[0:1, 0:1])

            psum_Ts = []
            for t in range(N_SEG):
                seg = slice(t * SEG, (t + 1) * SEG)
                pT = pT_pool.tile([128, LH], f32, tag="pT")
                nc.tensor.transpose(pT[:, 0:128], a1[:, seg], id128)
                nc.tensor.transpose(pT[:, 128:144], a2[:, seg], id16)
                psum_Ts.append(pT)

            # head means for all segments: hmall[p, 12t+l] = mean_h att
            hmall = wpool.tile([128, N_SEG * L], f32, tag="hmall")
            for t in range(N_SEG):
                nc.vector.tensor_reduce(
                    hmall[:, t * L:(t + 1) * L],
                    psum_Ts[t][:].rearrange("p (l h) -> p l h", h=H), axis=X,
                    op=Alu.add,
                )
            hms = wpool.tile([128, N_SEG * L], f32, tag="hms")
            nc.vector.tensor_scalar_mul(
                hms[:], hmall[:], float(np.float32(1.0 / 12.0)))

            # product over layers via batched pairwise tree: [128, 4, 12]
            hv = hms[:].rearrange("p (t l) -> p t l", l=L)
            p6 = wpool.tile([128, N_SEG * 6], f32, tag="p6")
            p6v = p6[:].rearrange("p (t l) -> p t l", l=6)
            nc.vector.tensor_tensor(p6v, hv[:, :, 0:6], hv[:, :, 6:12],
                                    op=Alu.mult)
            p3 = wpool.tile([128, N_SEG * 3], f32, tag="p3")
            p3v = p3[:].rearrange("p (t l) -> p t l", l=3)
            nc.vector.tensor_tensor(p3v, p6v[:, :, 0:3], p6v[:, :, 3:6],
                                    op=Alu.mult)
            p1 = wpool.tile([128, N_SEG], f32, tag="p1")
            p1v = p1[:].rearrange("p (t l) -> p t l", l=1)
            nc.vector.tensor_tensor(p1v, p3v[:, :, 0:1], p3v[:, :, 1:2],
                                    op=Alu.mult)
            prodall = wpool.tile([128, N_SEG], f32, tag="prodall")
            nc.vector.tensor_tensor(
                prodall[:].rearrange("p (t l) -> p t l", l=1), p1v,
                p3v[:, :, 2:3], op=Alu.mult)

            # packed output: y_soft columns = -prod
            out_s = rpool.tile([128, 2 * N_SEG], f32, tag="out")
            nc.vector.tensor_scalar_mul(out_s[:, 0:N_SEG], prodall[:], -1.0)

            # psel = prod with psel[0] = 1.0 (> any product of softmax means,
            # so CLS ranks first; degenerate inputs hit the host tie-fallback
            # keyed off y_soft).  Column form only:
            pcol0 = rpool.tile([128, 1], f32, tag="pcol0")
            nc.vector.tensor_copy(pcol0[:], prodall[:, 0:1])
            nc.vector.memset(pcol0[0:1, 0:1], 1.0)
            cols = [pcol0[:]] + [prodall[:, t:t + 1] for t in range(1, N_SEG)]

            # bc[p, i] = psel[i]: replicate each psel column along free, then
            # PE-transpose the segments into one PSUM tile (no row form, no
            # K=1 fp32 matmul needed)
            psum_bc = prow_pool.tile([128, S], f32, tag="bc")
            for t in range(N_SEG):
                rep = mpool.tile([128, SEG], f32, tag="rep")
                nc.vector.tensor_copy(rep[:], cols[t].broadcast_to([128, SEG]))
                nc.tensor.transpose(
                    psum_bc[:, t * SEG:(t + 1) * SEG], rep[:], id128)

            # strict rank[j] = #{i: psel[i] > psel[j]}; partition p = j-within-
            # segment (j = 128t+p), free f = i
            for t in range(N_SEG):
                gt = mpool.tile([128, S], f32, tag="gt")
                rank_t = wpool.tile([128, 1], f32, tag="rank")
                nc.vector.tensor_scalar(
                    gt[:], psum_bc[:], cols[t], None, op0=Alu.is_gt,
                    op1=Alu.add, accum_out=rank_t[:])
                nc.vector.tensor_scalar(
                    out_s[:, N_SEG + t:N_SEG + t + 1], rank_t[:], float(k),
                    None, op0=Alu.is_lt)

            nc.sync.dma_start(y_out[:], out_s[:])

    return nc


LAST_RESULT = None  # BassKernelResults of the most recent run (for profiling)


def _ensure_ntff_hook():
    """bass_utils hard-imports antenv.axon_hooks when tracing is requested;
    this container's antenv lacks it. Provide it (with a working hook when
    the axon .so supports NRT profiling)."""
    import sys
    import types

    try:
        import antenv.axon_hooks  # noqa: F401

        return
    except ImportError:
        pass
    mod = types.ModuleType("antenv.axon_hooks")
    state = [None]
    mod.set_axon_ntff_profile_hook = lambda h: state.__setitem__(0, h)
    mod.get_axon_ntff_profile_hook = lambda: state[0]
    try:
        from trn_agent_boot.trn_boot import _ntff_profile_via_ctypes

        state[0] = _ntff_profile_via_ctypes("/opt/axon/libaxon_pjrt.so")
    except Exception:
        pass
    try:
        import antenv

        antenv.axon_hooks = mod
    except ImportError:
        pass
    sys.modules["antenv.axon_hooks"] = mod


def _run(att_cls: np.ndarray, k: int):
    global LAST_RESULT
    _ensure_ntff_hook()
    from concourse.bass_utils import run_bass_kernel_spmd

    nc = _build(k)
    in_maps = [
        {"att": np.ascontiguousarray(att_cls[:, b].reshape(LH, S))}
        for b in range(B)
    ]
    LAST_RESULT = run_bass_kernel_spmd(nc, in_maps, list(range(N_CORES)))
    res = LAST_RESULT.results
    y_soft = np.stack([res[b]["y_out"][:, 0:N_SEG].T.reshape(S)
                       for b in range(B)])
    y_hard = np.stack([res[b]["y_out"][:, N_SEG:].T.reshape(S)
                       for b in range(B)]) > 0.5
    if any(np.unique(y_soft[b]).size != S for b in range(B)):
        # exact duplicate values: strict rank != stable rank; replicate the
        # reference's stable double-argsort on host (f32, global min)
        y = y_soft.copy()
        y[:, 0] = np.float32(y_soft.min() - np.float32(1.0))
        order = np.argsort(y, axis=-1, kind="stable")
        rank = np.argsort(order, axis=-1, kind="stable")
        y_hard = rank < k
    return y_hard, y_soft


def kernel(attentions, embedding_sequence, compression_rate):
    att = np.asarray(attentions)
    seq_len = int(np.asarray(embedding_sequence).shape[1])
    k = max(int(seq_len * (1.0 - float(np.asarray(compression_rate)))), 1)
    att_cls = np.ascontiguousarray(att[:, :, :, 0, :], dtype=np.float32)
    y_hard, y_soft = _run(att_cls, k)
    return y_hard, y_soft


# revision 33
# speedup vs baseline: 1.8744x; 1.0985x over previous
"""Trainium2 Bass kernel for BERTIdealEmissionRateCompressionModule.

reference math (teacher path):
    head_mean = attentions.mean(axis=2)          # [L, B, S, S]
    prod      = prod_L head_mean                 # [B, S, S]
    y_soft    = -prod[:, 0, :]                   # [B, S]   <- only CLS row used!
    y_hard    = rank(y_soft with y[0]=min-1) < k # [B, S] bool, stable ranking

Only attentions[:, :, :, 0, :] (L*H*S floats per batch row) is live data.
Sharding: pure data parallel over batch B=8 -> one batch row per NeuronCore.
The host also pre-transposes each core's slice to token-major [S, L*H] so the
device needs no layout work on the input.

Per-core device pipeline (attT [512, 144] f32, 144 = L*H):
  1. DMA attT -> SBUF as [128, 4*144] (token p, free = (seg t, l, h))
  2. one reduce over h -> sums [128, 4*12]; fused *1/144 into the first
     pairwise multiply; pairwise tree over l -> prodall [128, 4]
     (prodall[p, t] = prod of head-means for token j = 128t + p)
  3. y_soft columns = -prodall (packed output cols 0:4)
  4. psel = prod with psel[0] = 1.0 (1.0 > any product of softmax means, so
     CLS always ranks first; degenerate inputs hit the host tie-fallback)
  5. bc[p, i] = psel[i]: replicate each psel column along free on DVE, then
     PE-transpose segments into one PSUM tile (identity const, 1 small DMA)
  6. strict rank[j] = #{i: psel[i] > psel[j]} via fused is_gt + row-sum
     (tensor_scalar accum_out), one op per 128-token segment
  7. y_hard = rank < k -> packed output cols 4:8; single DMA out
Host: reorders columns to rows, casts mask to bool; if any y_soft row has
duplicate values (exact ties -- impossible for real attention products), the
mask is recomputed on host with the reference's stable double-argsort.
"""

import functools

import numpy as np

L, B, H, S = 12, 8, 12, 512
LH = L * H  # 144
N_CORES = 8
SEG = 128
N_SEG = S // SEG  # 4


@functools.lru_cache(maxsize=4)
def _build(k: int):
    import concourse.bass as bass
    import concourse.mybir as mybir
    from concourse.tile import TileContext
    from concourse.vector_clock import ScopedClock

    class TileContextSplitDrain(TileContext):
        """This walrus codegen fits a single embedded sync wait per
        instruction; Tile's kernel-tail drain aggregates one wait per live
        semaphore onto one Drain. Split it into a chain of single-wait
        drains on the sync queue (same semantics: all waits complete
        before the end-of-kernel barrier)."""

        def _drain_and_barrier(self, tick_clock, wait_clock):
            nc = self.nc
            drain_inst = nc.sync.drain()
            wait_clock.add_sem_waits(
                drain_inst.ins, ScopedClock({None: tick_clock.global_clock})
            )
            si = drain_inst.ins.sync_info
            if si is not None and len(si.on_wait) > 1:
                waits = list(si.on_wait)
                ups = list(si.on_update)
                drain_inst.ins.sync_info = mybir.SyncInfo(
                    on_wait=[waits[0]], on_update=[])
                for w in waits[1:-1]:
                    d = nc.sync.drain()
                    d.ins.sync_info = mybir.SyncInfo(on_wait=[w], on_update=[])
                d = nc.sync.drain()
                d.ins.sync_info = mybir.SyncInfo(
                    on_wait=[waits[-1]], on_update=ups)
            nc.all_engine_barrier()
            assert self.sems is not None
            popped = nc._tile_sem_poison_stack.pop()
            assert popped is self._sem_poison
            nc.clear_and_free_semaphores(list(self.sems.allocated().values()))
            nc.all_engine_barrier()

    f32 = mybir.dt.float32
    Alu = mybir.AluOpType
    X = mybir.AxisListType.X

    nc = bass.Bass()
    attT = nc.declare_dram_parameter("attT", [S, LH], f32, isOutput=False)
    # packed column-form output: cols 0:4 = y_soft, cols 4:8 = y_hard mask,
    # out[p, c] for token j = 128*(c%4) + p
    y_out = nc.dram_tensor("y_out", [SEG, 2 * N_SEG], f32,
                           kind="ExternalOutput")

    pack_d = nc.inline_tensor(np.eye(128, dtype=np.float32), "cpack")

    with TileContextSplitDrain(nc) as tc:
        with (
            tc.tile_pool(name="const", bufs=1) as cpool,
            tc.tile_pool(name="inp", bufs=1) as ipool,
            tc.tile_pool(name="work", bufs=2) as wpool,
            tc.tile_pool(name="rowbuf", bufs=1) as rpool,
            tc.tile_pool(name="cmp", bufs=N_SEG) as mpool,
            tc.tile_pool(name="pbc", bufs=1, space="PSUM") as pbc_pool,
            tc.tile_pool(name="pdum", bufs=1, space="PSUM") as pdum_pool,
        ):
            # input first on the SP queue; identity const in parallel on ACT
            at = ipool.tile([128, N_SEG * LH], f32, tag="at")
            nc.sync.dma_start(
                at[:].rearrange("p (t c) -> p t c", c=LH),
                attT[:].rearrange("(t p) c -> p t c", p=SEG))
            cpack = cpool.tile([128, 128], f32, tag="cpack")
            nc.scalar.dma_start(cpack[:], pack_d[:])
            id128 = cpack[:, 0:128]

            # dummy 1x1 transpose: makes the PE take the const-DMA wait
            # alone, so every later matmul carries at most one sync wait
            # (this walrus codegen fits a single wait per compute inst).
            pdum = pdum_pool.tile([1, 1], f32, tag="pdum")
            nc.tensor.transpose(pdum[:], cpack[0:1, 0:1], cpack[0:1, 0:1])

            # head sums over h for all segments: sums[p, (t, l)]
            sums = wpool.tile([128, N_SEG * L], f32, tag="sums")
            nc.vector.tensor_reduce(
                sums[:], at[:].rearrange("p (t l h) -> p t l h", l=L, h=H),
                axis=X, op=Alu.add)

            # product over layers, 1/12 mean scales folded pairwise:
            # p6 = (s_l / 144) * s_{l+6} == (s_l/12) * (s_{l+6}/12) up to 1ulp
            sv = sums[:].rearrange("p (t l) -> p t l", l=L)
            p6 = wpool.tile([128, N_SEG * 6], f32, tag="p6")
            p6v = p6[:].rearrange("p (t l) -> p t l", l=6)
            nc.vector.scalar_tensor_tensor(
                p6v, sv[:, :, 0:6], float(np.float32(1.0 / 144.0)),
                sv[:, :, 6:12], op0=Alu.mult, op1=Alu.mult)
            p3 = wpool.tile([128, N_SEG * 3], f32, tag="p3")
            p3v = p3[:].rearrange("p (t l) -> p t l", l=3)
            nc.vector.tensor_tensor(p3v, p6v[:, :, 0:3], p6v[:, :, 3:6],
                                    op=Alu.mult)
            p1 = wpool.tile([128, N_SEG], f32, tag="p1")
            p1v = p1[:].rearrange("p (t l) -> p t l", l=1)
            nc.vector.tensor_tensor(p1v, p3v[:, :, 0:1], p3v[:, :, 1:2],
                                    op=Alu.mult)
            prodall = wpool.tile([128, N_SEG], f32, tag="prodall")
            nc.vector.tensor_tensor(
                prodall[:].rearrange("p (t l) -> p t l", l=1), p1v,
                p3v[:, :, 2:3], op=Alu.mult)

            # packed output: y_soft columns = -prod
            out_s = rpool.tile([128, 2 * N_SEG], f32, tag="out")
            nc.vector.tensor_scalar_mul(out_s[:, 0:N_SEG], prodall[:], -1.0)

            # psel column form with CLS sentinel
            pcol0 = rpool.tile([128, 1], f32, tag="pcol0")
            nc.vector.tensor_copy(pcol0[:], prodall[:, 0:1])
            nc.vector.memset(pcol0[0:1, 0:1], 1.0)
            cols = [pcol0[:]] + [prodall[:, t:t + 1] for t in range(1, N_SEG)]

            # bc[p, i] = psel[i]: replicate columns along free, PE-transpose
            psum_bc = pbc_pool.tile([128, S], f32, tag="bc")
            for t in range(N_SEG):
                rep = mpool.tile([128, SEG], f32, tag="rep")
                nc.vector.tensor_copy(rep[:], cols[t].broadcast_to([128, SEG]))
                nc.tensor.transpose(
                    psum_bc[:, t * SEG:(t + 1) * SEG], rep[:], id128)

            # strict rank[j] = #{i: psel[i] > psel[j]}; partition p = j-within-
            # segment (j = 128t+p), free f = i
            for t in range(N_SEG):
                gt = mpool.tile([128, S], f32, tag="gt")
                rank_t = wpool.tile([128, 1], f32, tag="rank")
                nc.vector.tensor_scalar(
                    gt[:], psum_bc[:], cols[t], None, op0=Alu.is_gt,
                    op1=Alu.add, accum_out=rank_t[:])
                nc.vector.tensor_scalar(
                    out_s[:, N_SEG + t:N_SEG + t + 1], rank_t[:], float(k),
                    None, op0=Alu.is_lt)

            nc.sync.dma_start(y_out[:], out_s[:])

    return nc


LAST_RESULT = None  # BassKernelResults of the most recent run (for profiling)


def _ensure_ntff_hook():
    """bass_utils hard-imports antenv.axon_hooks when tracing is requested;
    this container's antenv lacks it. Provide it (with a working hook when
    the axon .so supports NRT profiling)."""
    import sys
    import types

    try:
        import antenv.axon_hooks  # noqa: F401

        return
    except ImportError:
        pass
    mod = types.ModuleType("antenv.axon_hooks")
    state = [None]
    mod.set_axon_ntff_profile_hook = lambda h: state.__setitem__(0, h)
    mod.get_axon_ntff_profile_hook = lambda: state[0]
    try:
        from trn_agent_boot.trn_boot import _ntff_profile_via_ctypes

        state[0] = _ntff_profile_via_ctypes("/opt/axon/libaxon_pjrt.so")
    except Exception:
        pass
    try:
        import antenv

        antenv.axon_hooks = mod
    except ImportError:
        pass
    sys.modules["antenv.axon_hooks"] = mod


def _run(attT_all: np.ndarray, k: int):
    global LAST_RESULT
    _ensure_ntff_hook()
    from concourse.bass_utils import run_bass_kernel_spmd

    nc = _build(k)
    in_maps = [{"attT": attT_all[b]} for b in range(B)]
    LAST_RESULT = run_bass_kernel_spmd(nc, in_maps, list(range(N_CORES)))
    res = LAST_RESULT.results
    y_soft = np.stack([res[b]["y_out"][:, 0:N_SEG].T.reshape(S)
                       for b in range(B)])
    y_hard = np.stack([res[b]["y_out"][:, N_SEG:].T.reshape(S)
                       for b in range(B)]) > 0.5
    if any(np.unique(y_soft[b]).size != S for b in range(B)):
        # exact duplicate values: strict rank != stable rank; replicate the
        # reference's stable double-argsort on host (f32, global min)
        y = y_soft.copy()
        y[:, 0] = np.float32(y_soft.min() - np.float32(1.0))
        order = np.argsort(y, axis=-1, kind="stable")
        rank = np.argsort(order, axis=-1, kind="stable")
        y_hard = rank < k
    return y_hard, y_soft


def kernel(attentions, embedding_sequence, compression_rate):
    att = np.asarray(attentions)
    seq_len = int(np.asarray(embedding_sequence).shape[1])
    k = max(int(seq_len * (1.0 - float(np.asarray(compression_rate)))), 1)
    # live data: CLS attention row only, token-major per batch: [B, S, L*H]
    attT_all = np.ascontiguousarray(
        att[:, :, :, 0, :].transpose(1, 3, 0, 2).reshape(B, S, LH),
        dtype=np.float32)
    y_hard, y_soft = _run(attT_all, k)
    return y_hard, y_soft
